# revision 10
# baseline (speedup 1.0000x reference)
"""Bilinear discriminator scores = sigmoid((x @ W.T) @ y.T) on 8 NeuronCores.

Sharding: rows of x (and of the output) split 8 ways; y and W replicated.
Per core: xt.T = W.T.T @ x.T via TensorE (K=d), then for each 128-row block
scores = sigmoid(xt.T.T @ y.T) with the contraction over d' on partitions.
All matmuls run in float32r (tf32-like, full-rate); sigmoid on ScalarE
straight out of PSUM with an fp16 store (halves output HBM traffic; sigmoid
outputs live in (0,1) where fp16 keeps ~1e-4 relative error), staged in SBUF
as full [128, 8192] row-blocks for 2 MiB DMA stores, upcast on host.

Host-side prep is layout only: transposes (x.T, y.T, W.T) and row-sharding.
"""

import numpy as np

import concourse.bass as bass
import concourse.tile as tile
from concourse import mybir
from concourse.bass_utils import run_bass_kernel_spmd

N, M, D = 8192, 8192, 256
NCORES = 8
NS = N // NCORES  # 1024 output rows per core
P = 128  # partitions
JBLK = 512  # matmul moving free dim (one PSUM bank of fp32)
JGRP = 2048  # sigmoid batch: 4 PSUM banks per ScalarE activation

_F32 = mybir.dt.float32
_F32R = mybir.dt.float32r
_F16 = mybir.dt.float16


def _split_multi_waits(nc):
    """This walrus build rejects >1 sync-wait per instruction; Tile emits
    several. Move extra waits onto same-engine NoOps inserted just before
    the instruction (same engine stream order => identical semantics)."""
    ctr = 0
    for func in nc.m.functions:
        for bb in func.blocks:
            out = []
            changed = False
            for inst in bb.instructions:
                si = getattr(inst, "sync_info", None)
                waits = list(si.on_wait) if si is not None and si.on_wait else []
                if len(waits) > 1:
                    changed = True
                    for w in waits[:-1]:
                        ctr += 1
                        out.append(
                            mybir.InstNoOp(
                                name=f"split_wait_nop_{ctr}",
                                sync_info=mybir.SyncInfo(on_wait=[w], on_update=[]),
                                bass_nofuse=True,
                                engine=inst.engine,
                            )
                        )
                    si.on_wait = [waits[-1]]
                out.append(inst)
            if changed:
                bb.instructions = out


def _emit(nc, tc, xT_ap, yT_ap, wT_ap, out_ap, repeats=1):
    import contextlib

    ctx = contextlib.ExitStack()
    with ctx:
        const = ctx.enter_context(tc.tile_pool(name="const", bufs=1))
        outp = ctx.enter_context(tc.tile_pool(name="outp", bufs=3))
        psum = ctx.enter_context(tc.tile_pool(name="psum", bufs=2, space="PSUM"))

        # ---- load inputs (d resp. d' on partitions). Small step-1 inputs
        # first so step 1 runs under the y load; y split into per-j-group
        # chunk tiles so step 2's early groups start before the tail lands.
        wT_sb = []
        xT_sb = []
        for dk in range(2):
            tw = const.tile([P, D], _F32R, name=f"w{dk}", tag=f"w{dk}")
            nc.sync.dma_start(tw[:], wT_ap[dk * P : (dk + 1) * P, :])
            wT_sb.append(tw)
            tx = const.tile([P, NS], _F32R, name=f"x{dk}", tag=f"x{dk}")
            nc.sync.dma_start(tx[:], xT_ap[dk * P : (dk + 1) * P, :])
            xT_sb.append(tx)
        # yT_sb[dk][g]: [P, JGRP] chunk for columns [g*JGRP, (g+1)*JGRP)
        yT_sb = [[None] * (M // JGRP) for _ in range(2)]
        for g in range(M // JGRP):
            for dk in range(2):
                ty = const.tile([P, JGRP], _F32R, name=f"y{dk}_{g}", tag=f"y{dk}_{g}")
                nc.sync.dma_start(
                    ty[:], yT_ap[dk * P : (dk + 1) * P, g * JGRP : (g + 1) * JGRP]
                )
                yT_sb[dk][g] = ty

        # ---- PE pre-warm: dummy matmuls on the (tiny, early-arriving) W
        # tile keep the PE busy during the y load so HAM un-throttles
        # (1.2 -> 2.4 GHz) before the real matmul stream starts. ----
        wps = psum.tile([P, JBLK], _F32, name="wps", tag="ps")
        for _ in range(16):
            nc.tensor.matmul(
                wps[:, 0:D],
                wT_sb[0][:, 0:P],
                wT_sb[0][:, 0:D],
                start=True,
                stop=True,
            )

        # ---- step 1: xtT[d', i] = sum_d W.T[d, d'] * xT[d, i] ----
        xtT_sb = [const.tile([P, NS], _F32R, name=f"xt{dp}", tag=f"xt{dp}") for dp in range(2)]
        for _rep in range(repeats):
          for dp in range(2):
            for ic2 in range(NS // JBLK):
                ps = psum.tile([P, JBLK], _F32, name="ps1", tag="ps")
                for dk in range(2):
                    nc.tensor.matmul(
                        ps[:],
                        wT_sb[dk][:, dp * P : (dp + 1) * P],
                        xT_sb[dk][:, ic2 * JBLK : (ic2 + 1) * JBLK],
                        start=(dk == 0),
                        stop=(dk == 1),
                    )
                nc.vector.tensor_copy(
                    xtT_sb[dp][:, ic2 * JBLK : (ic2 + 1) * JBLK], ps[:]
                )

          # ---- step 2: per 128-row block, scores then sigmoid then store ----
          for ic in range(NS // P):
            ob = outp.tile([P, M], _F16, name="ob", tag="ob")
            for jg in range(M // JGRP):
                ps = psum.tile([P, JGRP], _F32, name="ps2", tag="ps")
                for js in range(JGRP // JBLK):
                    for dp in range(2):
                        nc.tensor.matmul(
                            ps[:, js * JBLK : (js + 1) * JBLK],
                            xtT_sb[dp][:, ic * P : (ic + 1) * P],
                            yT_sb[dp][jg][:, js * JBLK : (js + 1) * JBLK],
                            start=(dp == 0),
                            stop=(dp == 1),
                        )
                nc.scalar.activation(
                    ob[:, jg * JGRP : (jg + 1) * JGRP],
                    ps[:],
                    mybir.ActivationFunctionType.Sigmoid,
                )
            nc.sync.dma_start(out_ap[ic * P : (ic + 1) * P, :], ob[:])


_built = {}


def _build(repeats=1):
    if repeats in _built:
        return _built[repeats]
    nc = bass.Bass("TRN2", target_bir_lowering=False, debug=False, num_devices=NCORES)
    xT_ap = nc.dram_tensor("xT", [D, NS], _F32R, kind="ExternalInput").ap()
    yT_ap = nc.dram_tensor("yT", [D, M], _F32R, kind="ExternalInput").ap()
    wT_ap = nc.dram_tensor("wT", [D, D], _F32R, kind="ExternalInput").ap()
    out_ap = nc.dram_tensor("out", [NS, M], _F16, kind="ExternalOutput").ap()
    with tile.TileContext(nc) as tc:
        _emit(nc, tc, xT_ap, yT_ap, wT_ap, out_ap, repeats=repeats)
    _split_multi_waits(nc)
    _built[repeats] = nc
    return nc


def kernel(x, y, W, **_unused):
    assert x.shape == (N, D) and y.shape == (M, D) and W.shape == (D, D)
    nc = _build()

    xT = np.ascontiguousarray(x.T.astype(np.float32, copy=False))
    yT = np.ascontiguousarray(y.T.astype(np.float32, copy=False))
    wT = np.ascontiguousarray(W.T.astype(np.float32, copy=False))

    in_maps = [
        {
            "xT": np.ascontiguousarray(xT[:, c * NS : (c + 1) * NS]),
            "yT": yT,
            "wT": wT,
        }
        for c in range(NCORES)
    ]
    res = run_bass_kernel_spmd(nc, in_maps, list(range(NCORES))).results
    out = np.empty((N, M), dtype=np.float32)
    for c in range(NCORES):
        out[c * NS : (c + 1) * NS, :] = res[c]["out"]
    return out
